# revision 31
# baseline (speedup 1.0000x reference)
"""Trainium2 Bass kernel for FConv2d (FFT conv module), v2.

out = irfftn( rfftn(x, axes=(c,h,w)) * rfftn(pad(weight)) )[:, :, ::4] reshaped.

Strategy (data-parallel over batch, 4 per core x 8 cores), all-bf16 data with
fp32 PSUM accumulation:
  S   joint 2D spatial rFFT while data is REAL (single pass): contract hw=1024
      via 8 PSUM-accumulated matmuls per batch; x is fed host-transposed as
      [hw, c] so no on-device transpose is needed.
  C   channel DFT (contract c=128) producing X[k_c, f] for 544 spatial freqs.
  MUL complex multiply with W-hat via Gauss 3-mult on DVE (+Pool offload),
      n-pair-wide ops with broadcast X operands.
  I1  subsampled inverse channel DFT via paired matmuls (bf16 rhs => full rate
      at N=64).
  I2  joint 2D spatial inverse + Re extraction via paired matmuls.
"""
import sys
import numpy as np

for _p in ("/opt/trn_rl_repo", "/root/.axon_site/_ro/trn_rl_repo"):
    if _p not in sys.path:
        sys.path.insert(0, _p)

import ml_dtypes

import concourse.bacc as bacc
import concourse.bass as bass
import concourse.mybir as mybir
import concourse.tile as tile
from concourse.bass_utils import run_bass_kernel_spmd

F32 = mybir.dt.float32
BF16 = mybir.dt.bfloat16
NPBF = ml_dtypes.bfloat16

B = 32          # full batch
B_LOC = 4       # per core
N_CORES = 8
CIN = 128
L = 32
NFIL = 8        # num filters n
NF = 544        # stored spatial freqs (32 * 17)
NFP = 640       # padded: 5 chunks of 128

# which n-pairs get their zr/zi add/sub on Pool instead of DVE
POOL_PAIRS = (0, 1, 2, 3)


# ----------------------------------------------------------------- constants
def build_constants(weight):
    f = np.arange(NF)
    p = f // 17
    q = f % 17

    # S: joint real 2D rfft factor, per hw chunk t: [128 hw, 1088]
    # cols = [Re(f) 0:544 | Im(f) 544:1088];  Re = cos, Im = -sin
    f2 = np.zeros((128, 8, 1088), dtype=np.float64)
    hw_p = np.arange(128)
    for t in range(8):
        h = 4 * t + hw_p // 32
        w = hw_p % 32
        ang = 2 * np.pi * (np.outer(h, p) + np.outer(w, q)) / 32.0
        f2[:, t, 0:NF] = np.cos(ang)
        f2[:, t, NF:2 * NF] = -np.sin(ang)

    # C: channel DFT lhsT pack [c, 384] = [cos | sin | -sin]
    c = np.arange(128)
    k = np.arange(128)
    angc = 2 * np.pi * np.outer(c, k) / 128.0
    fc = np.concatenate(
        [np.cos(angc), np.sin(angc), -np.sin(angc)], axis=1)

    # I1 rhs: e1 = [cos|sin], e2 = [-sin|cos] at output positions c=4j
    j32 = np.arange(32)
    ange = 2 * np.pi * np.outer(k, j32) / 32.0
    er = np.cos(ange)
    ei = np.sin(ange)
    e1 = np.concatenate([er, ei], axis=1)
    e2 = np.concatenate([-ei, er], axis=1)

    # I2 rhs: k2d[128, 5, 2, 1024]: cos / -sin of inverse angles
    k2d = np.zeros((128, 5, 2, 1024), dtype=np.float64)
    yz = np.arange(1024)
    y = yz // 32
    z = yz % 32
    pp = np.where(f < NF, p, 0)
    qq = np.where(f < NF, q, 0)
    fp = np.arange(NFP)
    valid = (fp < NF).astype(np.float64)
    pv = np.zeros(NFP, dtype=np.int64)
    qv = np.zeros(NFP, dtype=np.int64)
    pv[:NF] = p
    qv[:NF] = q
    for fcb in range(5):
        sl = slice(fcb * 128, (fcb + 1) * 128)
        ang4 = 2 * np.pi * (np.outer(pv[sl], y) + np.outer(qv[sl], z)) / 32.0
        k2d[:, fcb, 0, :] = np.cos(ang4) * valid[sl][:, None]
        k2d[:, fcb, 1, :] = -np.sin(ang4) * valid[sl][:, None]

    # W-hat folded with alpha/N normalization; Gauss pack per n-pair:
    # wpk[128, pair, kind, n_in_pair, 544]; kinds: A=wr, B=wr+wi, C=wi-wr
    w_hat = np.fft.rfftn(weight.astype(np.float64), s=(CIN, L, L),
                         axes=(1, 2, 3))
    alpha = np.full(17, 2.0)
    alpha[0] = 1.0
    alpha[16] = 1.0
    w_hat = w_hat * alpha[None, None, None, :] / (128.0 * 32.0 * 32.0)
    wr = np.transpose(w_hat.real, (1, 0, 2, 3)).reshape(128, NFIL, NF)
    wi = np.transpose(w_hat.imag, (1, 0, 2, 3)).reshape(128, NFIL, NF)
    wpk = np.zeros((128, 4, 3, 2, NF), dtype=np.float64)
    for pr in range(4):
        for o in range(2):
            n = 2 * pr + o
            wpk[:, pr, 0, o] = wr[:, n]
            wpk[:, pr, 1, o] = wr[:, n] + wi[:, n]
            wpk[:, pr, 2, o] = wi[:, n] - wr[:, n]

    cast = lambda a: a.astype(NPBF)
    return {
        "f2": cast(f2),
        "fc": cast(fc),
        "e1": cast(e1),
        "e2": cast(e2),
        "k2d": cast(k2d),
        "wpk": cast(wpk),
    }


# ----------------------------------------------------------------- program
def build_program(dbg=False):
    nc = bacc.Bacc("TRN2", target_bir_lowering=False, debug=False)
    x_d = nc.dram_tensor("xt", [B_LOC, 1024, 128], BF16, kind="ExternalInput")
    f2_d = nc.dram_tensor("f2", [128, 8, 1088], BF16, kind="ExternalInput")
    fc_d = nc.dram_tensor("fc", [128, 384], BF16, kind="ExternalInput")
    e1_d = nc.dram_tensor("e1", [128, 64], BF16, kind="ExternalInput")
    e2_d = nc.dram_tensor("e2", [128, 64], BF16, kind="ExternalInput")
    k2d_d = nc.dram_tensor("k2d", [128, 5, 2, 1024], BF16, kind="ExternalInput")
    wpk_d = nc.dram_tensor("wpk", [128, 4, 3, 2, NF], BF16, kind="ExternalInput")
    out_d = nc.dram_tensor("out", [B_LOC, 256, 32, 32], F32, kind="ExternalOutput")
    if dbg:
        dbg_xs = nc.dram_tensor("dbg_xs", [128, 1088], F32, kind="ExternalOutput")
        dbg_xc = nc.dram_tensor("dbg_xc", [128, 2, NF], F32, kind="ExternalOutput")
        dbg_z = nc.dram_tensor("dbg_z", [128, 2, 2, NFP], F32, kind="ExternalOutput")
        dbg_a = nc.dram_tensor("dbg_a", [128, 5, 2, NFIL, 32], F32, kind="ExternalOutput")

    with tile.TileContext(nc) as tc:
        with (
            tc.tile_pool(name="consts", bufs=1) as cpool,
            tc.tile_pool(name="xin", bufs=2) as xpool,
            tc.tile_pool(name="xs", bufs=2) as xspool,
            tc.tile_pool(name="xc", bufs=2) as xcpool,
            tc.tile_pool(name="z", bufs=2) as zpool,
            tc.tile_pool(name="a", bufs=1) as apool,
            tc.tile_pool(name="o", bufs=3) as opool,
            tc.tile_pool(name="ps_f", bufs=1, space="PSUM") as pfpool,
            tc.tile_pool(name="ps_i1", bufs=2, space="PSUM") as pi1pool,
            tc.tile_pool(name="ps_i2", bufs=2, space="PSUM") as pi2pool,
        ):
            # ---- constants, ordered by first use
            f2_sb = cpool.tile([128, 8, 1088], BF16)
            for t in range(8):
                nc.sync.dma_start(out=f2_sb[:, t], in_=f2_d[:, t])
            fc_sb = cpool.tile([128, 384], BF16)
            nc.sync.dma_start(out=fc_sb[:], in_=fc_d[:])
            e1_sb = cpool.tile([128, 64], BF16)
            nc.sync.dma_start(out=e1_sb[:], in_=e1_d[:])
            e2_sb = cpool.tile([128, 64], BF16)
            nc.sync.dma_start(out=e2_sb[:], in_=e2_d[:])
            wpk_sb = cpool.tile([128, 4, 3, 2, NF], BF16)
            for pr in range(4):
                nc.sync.dma_start(out=wpk_sb[:, pr], in_=wpk_d[:, pr])
            k2d_sb = cpool.tile([128, 5, 2, 1024], BF16)
            for fcb in range(5):
                nc.sync.dma_start(out=k2d_sb[:, fcb], in_=k2d_d[:, fcb])

            def emit_forward(b):
                # ---- load x[b] as [hw 128-part, (t, c)]
                xt = xpool.tile([128, 8, 128], BF16, tag="xt")
                nc.gpsimd.dma_start(
                    out=xt[:],
                    in_=x_d[b].rearrange("(t p) c -> p t c", p=128))

                # ---- S: joint real 2D rfft: Xs[c, 1088] in 3 PSUM tiles
                sp0 = pfpool.tile([128, 512], F32, tag="f0", name=f"sp0_{b}")
                sp1 = pfpool.tile([128, 512], F32, tag="f1", name=f"sp1_{b}")
                sp2 = pfpool.tile([128, 64], F32, tag="f2", name=f"sp2_{b}")
                for t in range(8):
                    st = (t == 0)
                    sp = (t == 7)
                    lhsT = xt[:, t, :]
                    nc.tensor.matmul(sp0[:], lhsT, f2_sb[:, t, 0:512],
                                     start=st, stop=sp)
                    nc.tensor.matmul(sp1[:], lhsT, f2_sb[:, t, 512:1024],
                                     start=st, stop=sp)
                    nc.tensor.matmul(sp2[:], lhsT, f2_sb[:, t, 1024:1088],
                                     start=st, stop=sp)
                xs_sb = xspool.tile([128, 1088], BF16, tag="xs")
                nc.scalar.copy(xs_sb[:, 0:512], sp0[:])
                nc.scalar.copy(xs_sb[:, 512:1024], sp1[:])
                nc.scalar.copy(xs_sb[:, 1024:1088], sp2[:])

                if dbg and b == 0:
                    xs_f = xspool.tile([128, 1088], F32, tag="xsf")
                    nc.gpsimd.tensor_scalar_mul(xs_f[:], xs_sb[:], 1.0)
                    nc.sync.dma_start(out=dbg_xs[:], in_=xs_f[:])

                # ---- C: channel DFT -> Xr/Xi [k_c, 544]
                # Xr = cosT@Xsr + sinT@Xsi ; Xi = cosT@Xsi - sinT@Xsr
                cp0 = pfpool.tile([128, 512], F32, tag="f0", name=f"cp0_{b}")
                cp1 = pfpool.tile([128, 512], F32, tag="f1", name=f"cp1_{b}")
                cp2 = pfpool.tile([128, 64], F32, tag="f2", name=f"cp2_{b}")
                lcos = fc_sb[:, 0:128]
                lsin = fc_sb[:, 128:256]
                lnsin = fc_sb[:, 256:384]
                nc.tensor.matmul(cp0[:], lcos, xs_sb[:, 0:512],
                                 start=True, stop=False)
                nc.tensor.matmul(cp0[:], lsin, xs_sb[:, 544:1056],
                                 start=False, stop=True)
                nc.tensor.matmul(cp2[:, 0:32], lcos, xs_sb[:, 512:544],
                                 start=True, stop=False, skip_group_check=True)
                nc.tensor.matmul(cp2[:, 0:32], lsin, xs_sb[:, 1056:1088],
                                 start=False, stop=True, skip_group_check=True)
                nc.tensor.matmul(cp1[:], lcos, xs_sb[:, 544:1056],
                                 start=True, stop=False)
                nc.tensor.matmul(cp1[:], lnsin, xs_sb[:, 0:512],
                                 start=False, stop=True)
                nc.tensor.matmul(cp2[:, 32:64], lcos, xs_sb[:, 1056:1088],
                                 start=True, stop=False, skip_group_check=True)
                nc.tensor.matmul(cp2[:, 32:64], lnsin, xs_sb[:, 512:544],
                                 start=False, stop=True, skip_group_check=True)

                xr = xcpool.tile([128, NF], BF16, tag="xr")
                xi = xcpool.tile([128, NF], BF16, tag="xi")
                xsum = xcpool.tile([128, NF], BF16, tag="xsum")
                nc.vector.tensor_scalar_mul(xr[:, 0:512], cp0[:], 1.0)
                nc.vector.tensor_scalar_mul(xr[:, 512:544], cp2[:, 0:32], 1.0)
                nc.vector.tensor_scalar_mul(xi[:, 0:512], cp1[:], 1.0)
                nc.vector.tensor_scalar_mul(xi[:, 512:544], cp2[:, 32:64], 1.0)
                nc.vector.tensor_add(xsum[:], xr[:], xi[:])

                if dbg and b == 0:
                    xc_f = xspool.tile([128, 2, NF], F32, tag="xcf")
                    nc.gpsimd.tensor_scalar_mul(xc_f[:, 0], xr[:], 1.0)
                    nc.gpsimd.tensor_scalar_mul(xc_f[:, 1], xi[:], 1.0)
                    nc.sync.dma_start(out=dbg_xc[:], in_=xc_f[:])

                # ---- MUL per n-pair (z for this b, consumed by back(b))
                zs = []
                for pr in range(4):
                    zr = zpool.tile([128, 2, NFP], BF16, tag=f"zr{pr}", bufs=3)
                    zi = zpool.tile([128, 2, NFP], BF16, tag=f"zi{pr}", bufs=3)
                    k1 = zpool.tile([128, 2, NF], BF16, tag=f"k1{pr}", bufs=2)
                    nc.gpsimd.memset(zr[:, :, NF:NFP].bitcast(F32), 0.0)
                    nc.gpsimd.memset(zi[:, :, NF:NFP].bitcast(F32), 0.0)
                    bc = lambda ap: ap.rearrange(
                        "p (o f) -> p o f", o=1).broadcast_to([128, 2, NF])
                    zrv = zr[:, :, 0:NF]
                    ziv = zi[:, :, 0:NF]
                    # zr = k1 - (wr+wi)*xi ; zi = k1 + (wi-wr)*xr
                    nc.vector.tensor_mul(ziv, bc(xr[:]), wpk_sb[:, pr, 2])
                    nc.vector.tensor_mul(zrv, bc(xi[:]), wpk_sb[:, pr, 1])
                    nc.vector.tensor_mul(k1[:], bc(xsum[:]), wpk_sb[:, pr, 0])
                    eng = nc.gpsimd if pr in POOL_PAIRS else nc.vector
                    eng.tensor_sub(zrv, k1[:], zrv)
                    eng.tensor_add(ziv, k1[:], ziv)
                    zs.append((zr, zi))

                    if dbg and b == 0 and pr == 0:
                        z_f = xspool.tile([128, 2, 2, NFP], F32, tag="zf")
                        nc.gpsimd.tensor_scalar_mul(z_f[:, 0], zr[:], 1.0)
                        nc.gpsimd.tensor_scalar_mul(z_f[:, 1], zi[:], 1.0)
                        nc.sync.dma_start(out=dbg_z[:], in_=z_f[:])
                return zs

            def emit_back(b, zs):
                # ---- I1 per n
                a_sb = apool.tile([128, 5, 2, NFIL, 32], BF16, tag="a")
                for pr in range(4):
                    zr, zi = zs[pr]
                    for o in range(2):
                        n = 2 * pr + o
                        ips = pi1pool.tile([128, 320], F32, tag="ips")
                        for fcb in range(5):
                            col = slice(fcb * 64, (fcb + 1) * 64)
                            zsl = slice(fcb * 128, (fcb + 1) * 128)
                            nc.tensor.matmul(
                                ips[:, col], zr[:, o, zsl], e1_sb[:],
                                start=True, stop=False)
                            nc.tensor.matmul(
                                ips[:, col], zi[:, o, zsl], e2_sb[:],
                                start=False, stop=True)
                        nc.scalar.copy(
                            a_sb[:, :, :, n, :],
                            ips[:].rearrange("p (fc c j) -> p fc c j",
                                             fc=5, c=2))

                if dbg and b == 0:
                    a_f = xspool.tile([128, 5, 2, NFIL, 32], F32, tag="af")
                    nc.gpsimd.tensor_scalar_mul(a_f[:], a_sb[:], 1.0)
                    nc.sync.dma_start(out=dbg_a[:], in_=a_f[:])

                # ---- I2: joint 2D inverse + Re extraction
                for mh in range(2):
                    for nzc in range(2):
                        ops = pi2pool.tile([128, 512], F32, tag="psi2")
                        for fcb in range(5):
                            for comp in range(2):
                                lhsT = a_sb[:, fcb, comp,
                                            mh * 4:(mh + 1) * 4, :]
                                rhs = k2d_sb[:, fcb, comp,
                                             nzc * 512:(nzc + 1) * 512]
                                nc.tensor.matmul(
                                    ops[:], lhsT, rhs,
                                    start=(fcb == 0 and comp == 0),
                                    stop=(fcb == 4 and comp == 1))
                        o_sb = opool.tile([128, 512], F32, tag="o")
                        nc.scalar.copy(o_sb[:], ops[:])
                        dst = out_d[b, mh * 128:(mh + 1) * 128].rearrange(
                            "c h w -> c (h w)")[:, nzc * 512:(nzc + 1) * 512]
                        nc.sync.dma_start(out=dst, in_=o_sb[:])

            # ---- software-pipelined schedule, depth 2:
            # fwd(b) runs two PE windows ahead of back(b)
            zs_all = {}
            DEPTH = 1
            for b in range(B_LOC):
                zs_all[b] = emit_forward(b)
                if b >= DEPTH:
                    emit_back(b - DEPTH, zs_all[b - DEPTH])
            for b in range(B_LOC - DEPTH, B_LOC):
                emit_back(b, zs_all[b])
    nc.compile()
    return nc


_CACHE = {}


def kernel(x, weight):
    x = np.asarray(x, dtype=np.float32)
    weight = np.asarray(weight, dtype=np.float32)
    consts = build_constants(weight)
    xt = np.ascontiguousarray(
        x.transpose(0, 2, 3, 1).reshape(B, 1024, 128)).astype(NPBF)
    if "nc" not in _CACHE:
        _CACHE["nc"] = build_program()
    nc = _CACHE["nc"]
    in_maps = []
    for i in range(N_CORES):
        m = {"xt": xt[i * B_LOC:(i + 1) * B_LOC]}
        m.update(consts)
        in_maps.append(m)
    res = run_bass_kernel_spmd(nc, in_maps, core_ids=list(range(N_CORES)))
    out = np.concatenate([r["out"] for r in res.results], axis=0)
    return out


if __name__ == "__main__":
    import jax

    sys.path.insert(0, "/root/problem")
    from reference import setup_inputs, reference

    with jax.default_device(jax.devices("cpu")[0]):
        inputs = setup_inputs()
        inputs = {k: np.asarray(v) for k, v in inputs.items()}
        expected = np.asarray(reference(**inputs))
    actual = kernel(**inputs)
    err = np.linalg.norm(actual - expected) / np.linalg.norm(expected)
    print("Relative error:", err)
